# revision 1
# baseline (speedup 1.0000x reference)
"""Trainium2 Bass kernel for a 2-layer GATv2 + dense-skip GNN (v2).

Architecture (dst-node parallel over 8 NeuronCores, as v1) with these
changes over the v1 baseline:

  - Gathers remain one indirect DMA per slot-column (TRN2 HW gathers one
    index per partition; multi-index offset APs are broken in HW), issued
    round-robin across `stripes` per-chunk w tiles. The ~1.78us/instruction
    GpSimd-queue cost (5.6ms total) is the kernel's hard floor.
  - The gather table is bf16 [N, 66]: 64 att-scaled xl columns plus the
    attention z-term split into two bf16 columns (z1=bf16(z),
    z2=bf16(z-z1)) so the softmax logit keeps ~fp32 accuracy.
  - The dst-side z term is dropped entirely: it is constant over each
    softmax segment and cancels.
  - Epilogue algebra: h = relu(numer/den + (xd - xr)) in scaled space;
    the 1/(0.4a) unscale is folded into the next layer's weights (relu
    commutes with a positive per-column scale). This removes the t2/inva
    work of v1.
  - Edge-phase ops run per equal-degree run of tiles (one instruction
    covers [P, R, d, 66]) instead of per tile; softmax exp runs per run
    with an explicit subtract (reduce negate=True gives -max directly).
  - x is fed to the device pre-transposed so layer-1 transforms need no
    PE transpose / identity.
  - Layer-2 transform folds the bias row into the matmul (k=65).
"""
import numpy as np

P = 128
H = 64
NCORES = 8
NEGBIG = 1.0e8
TW = 195           # transform cols: 64 xl | 1 z | 64 xr | 2 zero | 64 xdm
XL_END = 64
ZCOL = 64
XR_OFF = 65
XRD_OFF = 65       # xrd = ot[:, 65:195]: [64 xr][2 zero][64 xdm]
GTC = 66           # gather-table cols: 64 xl-scaled + z1 + z2


class Cfg:
    def __init__(self, N, F_IN, NLOC, d_t, offs, p1, p2, b3val,
                 sdc_max=224, tc_max=24, gather_slots=96,
                 safe_reduce=False, no_mixed=False, per_slot_gather=True,
                 debug_out=False, f32_table=False, stripes=4):
        self.safe_reduce = safe_reduce
        self.no_mixed = no_mixed
        self.per_slot_gather = per_slot_gather
        self.debug_out = debug_out
        self.f32_table = f32_table
        self.stripes = stripes
        self.N = N
        self.F_IN = F_IN
        self.NLOC = NLOC
        self.NT = NLOC // P
        self.NPAD = NCORES * NLOC
        self.d_t = d_t
        self.offs = offs
        self.SD = int(offs[-1])
        self.p1 = p1
        self.p2 = p2
        self.b3val = b3val
        self.DZ_LP = NLOC - 4          # dummy-zero local slot
        self.DN_LP = NLOC - 3          # dummy-neg local slot
        self.gather_slots = gather_slots
        # chunks: [t0, t1) with sum(d) <= sdc_max and t1-t0 <= tc_max
        chunks = []
        t0 = 0
        while t0 < self.NT:
            t1, acc = t0, 0
            while t1 < self.NT and t1 - t0 < tc_max and \
                    acc + d_t[t1] <= sdc_max:
                acc += d_t[t1]
                t1 += 1
            assert t1 > t0, f"tile degree {d_t[t0]} exceeds sdc_max"
            if stripes > 1 and t1 < self.NT:
                r = (t1 - t0) % stripes
                if r and t1 - t0 > r:
                    t1 -= r
            chunks.append((t0, t1))
            t0 = t1
        self.chunks = chunks
        # runs of equal d within each chunk: list of (t0, t1) per chunk
        self.runs = []
        for (c0, c1) in chunks:
            r = []
            t = c0
            while t < c1:
                u = t
                while u < c1 and d_t[u] == d_t[t]:
                    u += 1
                r.append((t, u))
                t = u
            self.runs.append(r)


# ---------------------------------------------------------------- host prep

def prep_graph(edge_index, N, NLOC):
    src = np.asarray(edge_index[0], dtype=np.int64)
    dst = np.asarray(edge_index[1], dtype=np.int64)
    NT = NLOC // P
    NPAD = NCORES * NLOC
    deg = np.bincount(dst, minlength=N)
    order = np.argsort(-deg, kind="stable")
    g_of_old = np.empty(N, dtype=np.int64)
    ii = np.arange(N)
    g_of_old[order] = (ii % NCORES) * NLOC + (ii // NCORES)
    deg_sorted = deg[order]
    d_t = [max(1, int(deg_sorted[NCORES * P * t])) if NCORES * P * t < N else 1
           for t in range(NT)]
    offs = np.concatenate([[0], np.cumsum(d_t)]).astype(np.int64)
    SD = int(offs[-1])

    dst_g = g_of_old[dst]
    es = np.argsort(dst_g, kind="stable")
    src_g_sorted = g_of_old[src[es]].astype(np.int32)
    dst_g_sorted = dst_g[es]
    node_start = np.searchsorted(dst_g_sorted, np.arange(NPAD), side="left")
    node_end = np.searchsorted(dst_g_sorted, np.arange(NPAD), side="right")

    DZ_LP, DN_LP = NLOC - 4, NLOC - 3
    esrc = np.zeros((NCORES, P, SD), dtype=np.int32)
    for c in range(NCORES):
        base = c * NLOC
        dz, dn = base + DZ_LP, base + DN_LP
        ns = node_start[base:base + NLOC]
        ne = node_end[base:base + NLOC]
        nd = ne - ns
        for t in range(NT):
            d = d_t[t]
            blk = np.full((P, d), dn, dtype=np.int32)
            for p in range(P):
                lp = t * P + p
                k = nd[lp]
                if k == 0:
                    blk[p, :] = dz
                else:
                    kk = min(k, d)
                    blk[p, :kk] = src_g_sorted[ns[lp]:ns[lp] + kk]
            esrc[c, :, offs[t]:offs[t + 1]] = blk
    return dict(order=order, d_t=d_t, offs=offs, SD=SD, esrc=esrc)


def prep_layer_weights(Wl, bl, Wr, br, att, b, Wd, bd,
                       in_perm=None, in_scale=None):
    """Fold one GATv2+dense layer into transform matrix Wt [K, 195] and
    bias row brow [195].

    Columns: [ (Wl*sc)[perm] | Wl@0.6att | (Wr*sc)[perm] | 0 0 |
               ((Wd-Wr)*sc)[perm] ]
    in_perm/in_scale: permutation+scale of the INPUT features (from the
    previous layer's output being in permuted, 0.4a-scaled space).
    Returns also inva (the positive per-column unscale to fold into the
    NEXT layer) and p_cnt.
    """
    Wl, bl, Wr, br, att, b, Wd, bd = [np.asarray(a, np.float64) for a in
                                      (Wl, bl, Wr, br, att, b, Wd, bd)]
    if in_perm is not None:
        Wl, Wr, Wd = Wl[in_perm], Wr[in_perm], Wd[in_perm]
        if in_scale is not None:
            Wl = Wl * in_scale[:, None]
            Wr = Wr * in_scale[:, None]
            Wd = Wd * in_scale[:, None]
    s = np.where(att >= 0, 1.0, -1.0)
    a = np.maximum(np.abs(att), 1e-12)
    perm = np.argsort(-s, kind="stable")
    p_cnt = int((s > 0).sum())
    sc = 0.4 * a
    scp = sc[perm]
    Wt = np.zeros((Wl.shape[0], TW), np.float64)
    Wt[:, 0:64] = (Wl * sc[None, :])[:, perm]
    Wt[:, ZCOL] = Wl @ (0.6 * att)
    Wt[:, 65:129] = (Wr * sc[None, :])[:, perm]
    Wt[:, 131:195] = ((Wd - Wr) * sc[None, :])[:, perm]
    brow = np.zeros((TW,), np.float64)
    brow[0:64] = (bl * sc)[perm]
    brow[ZCOL] = bl @ (0.6 * att)
    brow[65:129] = (br * sc)[perm]
    brow[131:195] = ((b + bd - br) * sc)[perm]
    inva = 1.0 / scp
    return dict(Wt=Wt.astype(np.float32),
                brow=brow.astype(np.float32)[None, :],
                inva=inva, perm=perm, p_cnt=p_cnt)


def dummy_rows_bf16(p_cnt):
    """Gather-table rows for dummy-zero (all 0) and dummy-neg nodes."""
    import ml_dtypes
    d = np.zeros((2, GTC), dtype=np.float32)
    d[1, p_cnt:64] = NEGBIG          # |w| huge in negative-sign columns
    d[1, 64] = -NEGBIG               # z1
    return d.astype(ml_dtypes.bfloat16)


# ------------------------------------------------------------- bass builder

def build_bass(cfg):
    import sys
    sys.path.insert(0, "/opt/trn_rl_repo")
    import concourse.bass as bass
    import concourse.bacc as bacc
    import concourse.mybir as mybir
    import concourse.tile as tile
    from concourse.masks import make_identity
    from contextlib import ExitStack

    f32 = mybir.dt.float32
    bf16 = mybir.dt.bfloat16
    if cfg.f32_table:
        bf16 = f32
    i32 = mybir.dt.int32
    X = mybir.AxisListType.X
    ADD = mybir.AluOpType.add
    SUB = mybir.AluOpType.subtract
    MULT = mybir.AluOpType.mult
    MAX = mybir.AluOpType.max
    AF = mybir.ActivationFunctionType

    NT, NLOC, NPAD, F_IN, SD = cfg.NT, cfg.NLOC, cfg.NPAD, cfg.F_IN, cfg.SD
    d_t, offs = cfg.d_t, cfg.offs

    nc = bacc.Bacc("TRN2", target_bir_lowering=False, num_devices=NCORES)

    xT_in = nc.dram_tensor("xT_loc", [F_IN, NLOC], f32, kind="ExternalInput")
    esrc_in = nc.dram_tensor("esrc", [P, SD], i32, kind="ExternalInput")
    wt1_in = nc.dram_tensor("wt1", [F_IN, TW], f32, kind="ExternalInput")
    brow1_in = nc.dram_tensor("brow1", [1, TW], f32, kind="ExternalInput")
    wt2_in = nc.dram_tensor("wt2", [H + 1, TW], f32, kind="ExternalInput")
    dummy1_in = nc.dram_tensor("dummy1", [2, GTC], bf16, kind="ExternalInput")
    dummy2_in = nc.dram_tensor("dummy2", [2, GTC], bf16, kind="ExternalInput")
    w3_in = nc.dram_tensor("w3rep", [P, H], f32, kind="ExternalInput")
    out3 = nc.dram_tensor("out3", [P, NT], f32, kind="ExternalOutput")
    if cfg.debug_out:
        dbg_xlb1 = nc.dram_tensor("dbg_xlb1", [NLOC, GTC], bf16,
                                  kind="ExternalOutput")
        dbg_xrd1 = nc.dram_tensor("dbg_xrd1", [P, NT, 130], f32,
                                  kind="ExternalOutput")
        dbg_full1 = nc.dram_tensor("dbg_full1", [NCORES * NLOC, GTC], bf16,
                                   kind="ExternalOutput")

    rg = [list(range(NCORES))]

    with ExitStack() as ctx:
        tc = ctx.enter_context(tile.TileContext(nc))
        dram = ctx.enter_context(tc.tile_pool(name="dram", bufs=1, space="DRAM"))
        xlb_loc = [dram.tile([NLOC, GTC], bf16, name=f"xlb{l}_loc")
                   for l in (1, 2)]
        xlb_full = [dram.tile([NPAD, GTC], bf16, name=f"xlb{l}_full",
                              addr_space="Shared") for l in (1, 2)]
        xrd_loc = [dram.tile([P, NT, 130], f32, name=f"xrd{l}_loc")
                   for l in (1, 2)]

        const = ctx.enter_context(tc.tile_pool(name="const", bufs=1))
        ident = const.tile([P, P], f32)
        make_identity(nc, ident[:, :])
        ones1 = const.tile([1, P], f32)
        nc.vector.memset(ones1[:, :], 1.0)
        wt1_s = const.tile([F_IN, TW], f32)
        nc.sync.dma_start(wt1_s[:, :], wt1_in[:, :])
        brow1_s = const.tile([1, TW], f32)
        nc.sync.dma_start(brow1_s[:, :], brow1_in[:, :])
        wt2_s = const.tile([H + 1, TW], f32)
        nc.sync.dma_start(wt2_s[:, :], wt2_in[:, :])
        dum_s = []
        for l, dum_i in enumerate([dummy1_in, dummy2_in]):
            dm = const.tile([2, GTC], bf16, name=f"dum{l}_s")
            nc.sync.dma_start(dm[:, :], dum_i[:, :])
            dum_s.append(dm)
        w3_s = const.tile([P, H], f32)
        nc.sync.dma_start(w3_s[:, :], w3_in[:, :])
        out3_s = const.tile([P, NT], f32)
        esrc_s = const.tile([P, SD], i32)
        nc.sync.dma_start(esrc_s[:, :], esrc_in[:, :])

        psT = ctx.enter_context(tc.tile_pool(name="psT", bufs=2, space="PSUM"))
        psM = ctx.enter_context(tc.tile_pool(name="psM", bufs=3, space="PSUM"))
        tp = ctx.enter_context(tc.tile_pool(name="tp", bufs=3))
        cp = ctx.enter_context(tc.tile_pool(name="cp", bufs=2))
        cw = ctx.enter_context(tc.tile_pool(name="cw", bufs=1))
        hp = ctx.enter_context(tc.tile_pool(name="hp", bufs=2))
        wp = ctx.enter_context(tc.tile_pool(name="wp", bufs=2))

        def transform_chunk_tail(ci, ot_chunk, layer):
            """ot_chunk [P, T_c, TW] f32 -> xlb/xrd writes for chunk ci."""
            t0, t1 = cfg.chunks[ci]
            T_c = t1 - t0
            xlb_c = cp.tile([P, T_c, GTC], bf16, tag="xlb_c")
            nc.scalar.copy(xlb_c[:, :, 0:64], ot_chunk[:, :, 0:64])
            nc.scalar.copy(xlb_c[:, :, 64:65], ot_chunk[:, :, ZCOL:ZCOL + 1])
            zr = cp.tile([P, T_c], f32, tag="zr")
            if cfg.no_mixed:
                z1f = cp.tile([P, T_c], f32, tag="z1f")
                nc.scalar.copy(z1f[:, :], xlb_c[:, :, 64:65].squeeze(2))
                nc.vector.tensor_tensor(
                    zr[:, :], ot_chunk[:, :, ZCOL:ZCOL + 1].squeeze(2),
                    z1f[:, :], SUB)
            else:
                nc.vector.tensor_tensor(
                    zr[:, :], ot_chunk[:, :, ZCOL:ZCOL + 1].squeeze(2),
                    xlb_c[:, :, 64:65].squeeze(2), SUB)
            nc.scalar.copy(xlb_c[:, :, 65:66], zr[:, :].unsqueeze(2))
            nc.sync.dma_start(
                xlb_loc[layer][t0 * P:t1 * P, :].rearrange(
                    "(t p) c -> p t c", p=P),
                xlb_c[:, :, :])
            nc.sync.dma_start(xrd_loc[layer][:, t0:t1, :],
                              ot_chunk[:, :, XRD_OFF:TW])

        # ---- phase T1: layer-1 transforms from xT (pre-transposed on host)
        for ci, (c0, c1) in enumerate(cfg.chunks):
            T_c = c1 - c0
            ot_chunk = cp.tile([P, T_c, TW], f32, tag="ot")
            t = c0
            while t < c1:
                q = min(4, c1 - t)
                xst = tp.tile([F_IN, 4 * P], f32, tag="lhsT")
                nc.sync.dma_start(xst[:, 0:q * P],
                                  xT_in[:, t * P:(t + q) * P])
                for i in range(q):
                    pm = psM.tile([P, TW], f32, tag="pm")
                    nc.tensor.matmul(pm[:, :], lhsT=xst[:, i * P:(i + 1) * P],
                                     rhs=wt1_s[:, :], start=True, stop=False)
                    nc.tensor.matmul(pm[:, :], lhsT=ones1[:, :],
                                     rhs=brow1_s[:, :], start=False, stop=True)
                    nc.scalar.copy(ot_chunk[:, t + i - c0, :], pm[:, :])
                t += q
            transform_chunk_tail(ci, ot_chunk, 0)

        def finish_layer_tables(layer):
            r0 = cfg.DZ_LP
            nc.sync.dma_start(xlb_loc[layer][r0:r0 + 2, :], dum_s[layer][:, :])
            nc.gpsimd.collective_compute(
                "AllGather", mybir.AluOpType.bypass, replica_groups=rg,
                ins=[xlb_loc[layer][:, :].opt()],
                outs=[xlb_full[layer][:, :].opt()])

        finish_layer_tables(0)

        # ---- edge phases
        def edge_phase(layer, p_cnt):
            xrd_tiles = {}

            def load_xrd(cj):
                if cj < len(cfg.chunks) and cj not in xrd_tiles:
                    b0, b1 = cfg.chunks[cj]
                    xt = cp.tile([P, b1 - b0, 130], f32, tag="xrd")
                    nc.sync.dma_start(xt[:, :, :],
                                      xrd_loc[layer][:, b0:b1, :])
                    xrd_tiles[cj] = xt

            for ci, (c0, c1) in enumerate(cfg.chunks):
                T_c = c1 - c0
                S = cfg.stripes if (cfg.stripes > 1
                                    and T_c % cfg.stripes == 0) else 1
                U = T_c // S
                load_xrd(ci)
                load_xrd(ci + 1)
                xrd = xrd_tiles.pop(ci)
                if S == 1:
                    xrd_sv = [xrd[:, :, :]]
                else:
                    xrd_sv = [xrd[:, :, :].rearrange(
                        "p (u s) c -> p s u c", s=S)[:, si, :, :]
                        for si in range(S)]

                den = cw.tile([P, T_c], f32, tag="den")
                emax = cw.tile([P, T_c], f32, tag="emax")
                numer = cw.tile([P, T_c, H], f32, tag="numer")

                def dview(full, si, u0, u1):
                    if S == 1:
                        return full[:, u0:u1]
                    return full[:, :].rearrange(
                        "p (u s) -> p s u", s=S)[:, si, u0:u1]

                def nview(si, u0, u1):
                    if S == 1:
                        return numer[:, u0:u1, :]
                    return numer[:, :, :].rearrange(
                        "p (u s) h -> p s u h", s=S)[:, si, u0:u1, :]

                stl = []
                for si in range(S):
                    ts = [c0 + u * S + si for u in range(U)]
                    ds = [d_t[t] for t in ts]
                    soffs = [0]
                    for dd in ds:
                        soffs.append(soffs[-1] + dd)
                    SDs = soffs[-1]
                    cols = [int(offs[t]) + c for u, t in enumerate(ts)
                            for c in range(ds[u])]
                    runs = []
                    u = 0
                    while u < U:
                        v = u
                        while v < U and ds[v] == ds[u]:
                            v += 1
                        runs.append((u, v))
                        u = v
                    stl.append(dict(
                        ds=ds, soffs=soffs, SDs=SDs, cols=cols, runs=runs,
                        w=wp.tile([P, SDs, GTC], bf16, tag=f"w{si}",
                                  name=f"w{si}"),
                        e=cw.tile([P, SDs], f32, tag=f"e{si}",
                                  name=f"e{si}"),
                        en=cw.tile([P, SDs], f32, tag=f"en{si}",
                                   name=f"en{si}"),
                        ex=cw.tile([P, SDs], bf16, tag=f"ex{si}",
                                   name=f"ex{si}")))

                # prefill each stripe's w with [xr | 0 0] broadcast, per run
                for si, st in enumerate(stl):
                    for (u0, u1) in st["runs"]:
                        d = st["ds"][u0]
                        a0, a1 = st["soffs"][u0], st["soffs"][u1]
                        nc.scalar.copy(
                            st["w"][:, a0:a1, :].rearrange(
                                "p (r d) c -> p r d c", d=d),
                            xrd_sv[si][:, u0:u1, 0:GTC].unsqueeze(2)
                            .to_broadcast([P, u1 - u0, d, GTC]))
                # gather-add xl rows; round-robin across stripes so several
                # indirect DMAs are in flight (WAW serializes per-tile)
                for k in range(max(st["SDs"] for st in stl)):
                    for st in stl:
                        if k < st["SDs"]:
                            col = st["cols"][k]
                            nc.gpsimd.indirect_dma_start(
                                out=st["w"][:, k, :], out_offset=None,
                                in_=xlb_full[layer][:, :],
                                in_offset=bass.IndirectOffsetOnAxis(
                                    ap=esrc_s[:, col:col + 1], axis=0),
                                compute_op=ADD)
                # logits + softmax + weighted aggregation, per stripe-run
                for si, st in enumerate(stl):
                    for (u0, u1) in st["runs"]:
                        d = st["ds"][u0]
                        R = u1 - u0
                        a0, a1 = st["soffs"][u0], st["soffs"][u1]
                        wr = st["w"][:, a0:a1, :].rearrange(
                            "p (r d) c -> p r d c", d=d)
                        er = st["e"][:, a0:a1].rearrange(
                            "p (r d) -> p r d", d=d)
                        exr = st["ex"][:, a0:a1].rearrange(
                            "p (r d) -> p r d", d=d)
                        emaxr = dview(emax, si, u0, u1)
                        denr = dview(den, si, u0, u1)
                        # e = sum(|w_pos|) - sum(|w_neg|) + z1 + z2
                        if p_cnt == 0:
                            nc.vector.tensor_reduce(
                                er, wr[:, :, :, 0:64], X, ADD,
                                apply_absolute_value=True, negate=True)
                        else:
                            nc.vector.tensor_reduce(
                                er, wr[:, :, :, 0:p_cnt], X, ADD,
                                apply_absolute_value=True)
                        if 0 < p_cnt < 64:
                            enr = st["en"][:, a0:a1].rearrange(
                                "p (r d) -> p r d", d=d)
                            nc.vector.tensor_reduce(
                                enr, wr[:, :, :, p_cnt:64], X, ADD,
                                apply_absolute_value=True)
                            nc.vector.tensor_tensor(er, er, enr, SUB)
                        nc.vector.tensor_tensor(
                            er, er, wr[:, :, :, 64:65].squeeze(3), ADD)
                        nc.vector.tensor_tensor(
                            er, er, wr[:, :, :, 65:66].squeeze(3), ADD)
                        # softmax over d (negate=True gives -max directly)
                        nc.vector.tensor_reduce(
                            emaxr, er, X, MAX, negate=True)
                        nc.vector.tensor_tensor(
                            er, er,
                            emaxr.unsqueeze(2).to_broadcast([P, R, d]), ADD)
                        nc.scalar.activation(exr, er, AF.Exp)
                        nc.vector.tensor_reduce(denr, exr, X, ADD)
                        # weighted sum: w[:, :, 0:64] *= ex ; reduce over d
                        nc.vector.tensor_tensor(
                            wr[:, :, :, 0:64], wr[:, :, :, 0:64],
                            exr.unsqueeze(3).to_broadcast([P, R, d, 64]),
                            MULT)
                        nc.vector.tensor_reduce(
                            nview(si, u0, u1),
                            wr[:, :, :, 0:64].transpose([0, 1, 3, 2]),
                            X, ADD)
                # h = relu(numer / den + xdm)   [P, T_c, 64]
                rden = cw.tile([P, T_c], f32, tag="rden")
                nc.vector.reciprocal(rden[:, :], den[:, :])
                nc.vector.tensor_tensor(
                    numer[:, :, :], numer[:, :, :],
                    rden[:, :].unsqueeze(2).to_broadcast([P, T_c, H]), MULT)
                nc.vector.tensor_tensor(
                    numer[:, :, :], numer[:, :, :], xrd[:, :, 66:130], ADD)
                h = hp.tile([P, T_c, H], f32, tag="h")
                nc.scalar.activation(h[:, :, :], numer[:, :, :], AF.Relu)

                if layer == 0:
                    # layer-2 transform for this chunk's tiles
                    ot_chunk = cp.tile([P, T_c, TW], f32, tag="ot2")
                    for t in range(c0, c1):
                        pt = psT.tile([H, P], f32, tag="pt")
                        nc.tensor.transpose(pt[:, :], h[:, t - c0, :],
                                            ident[:, :])
                        hT = tp.tile([H + 1, P], f32, tag="hT")
                        nc.scalar.copy(hT[0:H, :], pt[:, :])
                        nc.vector.memset(hT[H:H + 1, :], 1.0)
                        pm = psM.tile([P, TW], f32, tag="pm2")
                        nc.tensor.matmul(pm[:, :], lhsT=hT[:, :],
                                         rhs=wt2_s[:, :], start=True,
                                         stop=True)
                        nc.scalar.copy(ot_chunk[:, t - c0, :], pm[:, :])
                    transform_chunk_tail(ci, ot_chunk, 1)
                else:
                    # out3 column = sum_h h * w3 (product reuses numer)
                    nc.vector.tensor_tensor(
                        numer[:, :, :], h[:, :, :],
                        w3_s[:, :].unsqueeze(1).to_broadcast([P, T_c, H]),
                        MULT)
                    nc.vector.tensor_reduce(out3_s[:, c0:c1], numer[:, :, :],
                                            X, ADD)

            if layer == 0:
                finish_layer_tables(1)

        edge_phase(0, cfg.p1)
        edge_phase(1, cfg.p2)

        nc.vector.tensor_scalar(out3_s[:, :], out3_s[:, :], float(cfg.b3val),
                                None, ADD)
        nc.sync.dma_start(out3[:, :], out3_s[:, :])
        if cfg.debug_out:
            nc.sync.dma_start(dbg_xlb1[:, :], xlb_loc[0][:, :])
            nc.sync.dma_start(dbg_xrd1[:, :, :], xrd_loc[0][:, :, :])
            nc.sync.dma_start(dbg_full1[:, :], xlb_full[0][:, :])

    nc.finalize()
    return nc


# ------------------------------------------------------------------ kernel

def make_inputs_and_cfg(inputs, N, F_IN, NLOC, **cfg_kw):
    import ml_dtypes
    g = prep_graph(inputs["edge_index"], N, NLOC)
    w1 = prep_layer_weights(inputs["Wl1"], inputs["bl1"], inputs["Wr1"],
                            inputs["br1"], inputs["att1"], inputs["b1"],
                            inputs["Wd1"], inputs["bd1"])
    w2 = prep_layer_weights(inputs["Wl2"], inputs["bl2"], inputs["Wr2"],
                            inputs["br2"], inputs["att2"], inputs["b2"],
                            inputs["Wd2"], inputs["bd2"],
                            in_perm=w1["perm"], in_scale=w1["inva"])
    f32t = bool(cfg_kw.get("f32_table"))
    x = np.ascontiguousarray(np.asarray(inputs["x"], np.float32))
    W3p = np.asarray(inputs["W3"], np.float64)[w2["perm"]] * \
        w2["inva"][:, None]                                     # [H, 1]
    b3val = float(np.asarray(inputs["b3"], np.float32)[0])
    cfg = Cfg(N, F_IN, NLOC, g["d_t"], g["offs"], w1["p_cnt"], w2["p_cnt"],
              b3val, **cfg_kw)

    w3rep = np.broadcast_to(W3p[:, 0][None, :].astype(np.float32),
                            (P, H)).copy()
    d1 = dummy_rows_bf16(w1["p_cnt"])
    d2 = dummy_rows_bf16(w2["p_cnt"])
    if f32t:
        d1 = d1.astype(np.float32)
        d2 = d2.astype(np.float32)
    # wt2 with bias folded as row H (lhsT row of ones)
    wt2f = np.concatenate([w2["Wt"], w2["brow"]], axis=0)

    in_maps = []
    order = g["order"]
    for c in range(NCORES):
        ii = np.arange(c, N, NCORES)
        lp = ii // NCORES
        x_loc = np.zeros((NLOC, F_IN), dtype=np.float32)
        x_loc[lp] = x[order[ii]]
        in_maps.append({
            "xT_loc": np.ascontiguousarray(x_loc.T),
            "esrc": np.ascontiguousarray(g["esrc"][c]),
            "wt1": w1["Wt"], "brow1": w1["brow"],
            "wt2": wt2f,
            "dummy1": d1, "dummy2": d2,
            "w3rep": w3rep,
        })
    return cfg, in_maps, g


def unshard_output(results, g, N, NLOC):
    out = np.zeros((N, 1), dtype=np.float32)
    order = g["order"]
    for c in range(NCORES):
        o = np.asarray(results[c]["out3"])          # [128, NT]
        ii = np.arange(c, N, NCORES)
        lp = ii // NCORES
        out[order[ii], 0] = o[lp % P, lp // P]
    return out


def kernel(**inputs):
    import sys
    sys.path.insert(0, "/opt/trn_rl_repo")
    from concourse.bass_utils import run_bass_kernel_spmd
    N, F_IN, NLOC = 100000, 128, 12544
    cfg, in_maps, g = make_inputs_and_cfg(inputs, N, F_IN, NLOC,
                                          per_slot_gather=True)
    nc = build_bass(cfg)
    res = run_bass_kernel_spmd(nc, in_maps, core_ids=list(range(NCORES)))
    return unshard_output(res.results, g, N, NLOC)



# revision 8
# speedup vs baseline: 1.6900x; 1.6900x over previous
"""Trainium2 Bass kernel for a 2-layer GATv2 + dense-skip GNN (v3).

v3 replaces v2's per-slot-column indirect DMAs (1.78us each on the Pool
engine's SWDGE, 5.6ms total) with batched `dma_gather` instructions:

  - The per-layer node table is [NPAD/4, 512B]: 4 nodes per row, each node
    64 bf16 columns (0.4|a|-scaled xl, sign-permuted).  Pack-row indices
    fit int16 (25088 < 32767), which dma_gather requires.
  - One dma_gather per <=64-slot-column segment gathers 128*cols edges in
    a single Pool instruction (~2-8ns/descriptor vs 14ns/descriptor for
    v2's one-column indirect DMAs), round-robined over 4 SWDGE queues so
    descriptor generation runs on all four Q7 cpu pairs concurrently.
  - The 1-of-4 sub-row select is done with one scalar-engine copy plus
    three DVE copy_predicated passes using host-precomputed masks.
  - The z attention term is no longer stored in the table: with w = t + r
    (r = dst transform), e = sum_pos|w| - sum_neg|w| + 1.5*(sum_pos w -
    sum_neg w) equals the GATv2 logit up to a per-dst-segment constant
    (which softmax cancels), so z comes from two extra reductions.
  - Epilogue algebra as v2: h = relu(numer/den + (xd - xr)) in scaled
    space; the 1/(0.4a) unscale is folded into the next layer's weights.
"""
import numpy as np

P = 128
H = 64
NCORES = 8
NEGBIG = 1.0e8
TW = 192           # transform cols: 64 t | 64 xr | 64 xdm
GSEG = 28          # max slot-columns per dma_gather (3584 idxs)
WPAD = 66          # w tile inner stride (pad past H=64 to keep APs 3-D)
NQ = 4             # SWDGE queues


class Cfg:
    def __init__(self, N, F_IN, NLOC, d_t, offs, p1, p2, b3val,
                 sdc_max=112, tc_max=12, stripes=4, gt_bufs=4):
        self.N = N
        self.F_IN = F_IN
        self.NLOC = NLOC
        self.NT = NLOC // P
        self.NPAD = NCORES * NLOC
        self.d_t = d_t
        self.offs = offs
        self.SD = int(offs[-1])
        self.p1 = p1
        self.p2 = p2
        self.b3val = b3val
        self.stripes = stripes
        self.gt_bufs = gt_bufs
        self.DZ_LP = NLOC - 4          # dummy-zero local slot
        self.DN_LP = NLOC - 3          # dummy-neg local slot
        # chunks: [t0, t1) with sum(d) <= sdc_max and t1-t0 <= tc_max
        chunks = []
        t0 = 0
        while t0 < self.NT:
            t1, acc = t0, 0
            while t1 < self.NT and t1 - t0 < tc_max and \
                    acc + d_t[t1] <= sdc_max:
                acc += d_t[t1]
                t1 += 1
            assert t1 > t0, f"tile degree {d_t[t0]} exceeds sdc_max"
            if stripes > 1 and t1 < self.NT:
                r = (t1 - t0) % stripes
                if r and t1 - t0 > r:
                    t1 -= r
            chunks.append((t0, t1))
            t0 = t1
        self.chunks = chunks
        # per-chunk stripe plan (also consumed by the host index builder)
        self.plan = []
        goff = 0            # running col offset into the call-ordered stream
        for (c0, c1) in chunks:
            T_c = c1 - c0
            S = stripes if (stripes > 1 and T_c % stripes == 0) else 1
            U = T_c // S
            stl = []
            for si in range(S):
                ts = [c0 + u * S + si for u in range(U)]
                ds = [d_t[t] for t in ts]
                soffs = [0]
                for dd in ds:
                    soffs.append(soffs[-1] + dd)
                cols = [int(offs[t]) + c for u, t in enumerate(ts)
                        for c in range(ds[u])]
                runs = []
                u = 0
                while u < U:
                    v = u
                    while v < U and ds[v] == ds[u]:
                        v += 1
                    runs.append((u, v))
                    u = v
                # gather segments of <= GSEG columns
                segs = []
                a = 0
                while a < len(cols):
                    b = min(a + GSEG, len(cols))
                    segs.append((a, b, goff))
                    goff += b - a
                    a = b
                stl.append(dict(ds=ds, soffs=soffs, SDs=soffs[-1], cols=cols,
                                runs=runs, segs=segs, S=S, U=U))
            self.plan.append(stl)
        self.GCOLS = goff   # total columns in call-ordered stream (== SD)


# ---------------------------------------------------------------- host prep

def prep_graph(edge_index, N, NLOC):
    src = np.asarray(edge_index[0], dtype=np.int64)
    dst = np.asarray(edge_index[1], dtype=np.int64)
    NT = NLOC // P
    NPAD = NCORES * NLOC
    deg = np.bincount(dst, minlength=N)
    order = np.argsort(-deg, kind="stable")
    g_of_old = np.empty(N, dtype=np.int64)
    ii = np.arange(N)
    g_of_old[order] = (ii % NCORES) * NLOC + (ii // NCORES)
    deg_sorted = deg[order]
    d_t = [max(1, int(deg_sorted[NCORES * P * t])) if NCORES * P * t < N else 1
           for t in range(NT)]
    offs = np.concatenate([[0], np.cumsum(d_t)]).astype(np.int64)
    SD = int(offs[-1])

    dst_g = g_of_old[dst]
    es = np.argsort(dst_g, kind="stable")
    src_g_sorted = g_of_old[src[es]].astype(np.int32)
    dst_g_sorted = dst_g[es]
    node_start = np.searchsorted(dst_g_sorted, np.arange(NPAD), side="left")
    node_end = np.searchsorted(dst_g_sorted, np.arange(NPAD), side="right")

    DZ_LP, DN_LP = NLOC - 4, NLOC - 3
    esrc = np.zeros((NCORES, P, SD), dtype=np.int32)
    for c in range(NCORES):
        base = c * NLOC
        dz, dn = base + DZ_LP, base + DN_LP
        ns = node_start[base:base + NLOC]
        ne = node_end[base:base + NLOC]
        nd = ne - ns
        for t in range(NT):
            d = d_t[t]
            blk = np.full((P, d), dn, dtype=np.int32)
            for p in range(P):
                lp = t * P + p
                k = nd[lp]
                if k == 0:
                    blk[p, :] = dz
                else:
                    kk = min(k, d)
                    blk[p, :kk] = src_g_sorted[ns[lp]:ns[lp] + kk]
            esrc[c, :, offs[t]:offs[t + 1]] = blk
    return dict(order=order, d_t=d_t, offs=offs, SD=SD, esrc=esrc)


def build_gidx_masks(cfg, esrc_c):
    """Per-core wrapped int16 pack indices + sub-select masks, in
    (chunk, stripe, segment, col) call order.

    gidx: [128, 8*GCOLS] int16 — idx j of a call at wrapped position
          (j%16, callbase*8 + j//16), replicated across 16-partition groups.
    msks: [3, P, GCOLS] bf16 — 1.0 where sub == k+1 (call-ordered columns).
    """
    G = cfg.GCOLS
    gidx = np.zeros((128, 8 * G), dtype=np.int16)
    msks = np.zeros((3, P, G), dtype=np.uint8)
    for stl in cfg.plan:
        for st in stl:
            cols = st["cols"]
            for (a, b, goff) in st["segs"]:
                seg = esrc_c[:, [cols[k] for k in range(a, b)]]  # [P, b-a]
                packs = (seg >> 2).astype(np.int16)              # [P, cols]
                subs = (seg & 3).astype(np.int64)
                # idx j = k*128 + p -> value packs[p, k]
                idx1d = packs.T.reshape(-1)                      # [(b-a)*128]
                w = idx1d.reshape(-1, 16).T                      # [16, (b-a)*8]
                blk = np.tile(w, (8, 1))                         # [128, ...]
                gidx[:, 8 * goff:8 * (goff + (b - a))] = blk
                for k in range(1, 4):
                    msks[k - 1, :, goff:goff + (b - a)] = (subs == k)
    return gidx, msks.astype(np.uint8)


def prep_layer_weights(Wl, bl, Wr, br, att, b, Wd, bd,
                       in_perm=None, in_scale=None):
    """Fold one GATv2+dense layer into transform matrix Wt [K, 192] and
    bias row brow [192].

    Columns: [ (Wl*sc)[perm] | (Wr*sc)[perm] | ((Wd-Wr)*sc)[perm] ]
    """
    Wl, bl, Wr, br, att, b, Wd, bd = [np.asarray(a, np.float64) for a in
                                      (Wl, bl, Wr, br, att, b, Wd, bd)]
    if in_perm is not None:
        Wl, Wr, Wd = Wl[in_perm], Wr[in_perm], Wd[in_perm]
        if in_scale is not None:
            Wl = Wl * in_scale[:, None]
            Wr = Wr * in_scale[:, None]
            Wd = Wd * in_scale[:, None]
    s = np.where(att >= 0, 1.0, -1.0)
    a = np.maximum(np.abs(att), 1e-12)
    perm = np.argsort(-s, kind="stable")
    p_cnt = int((s > 0).sum())
    sc = 0.4 * a
    scp = sc[perm]
    Wt = np.zeros((Wl.shape[0], TW), np.float64)
    Wt[:, 0:64] = (Wl * sc[None, :])[:, perm]
    Wt[:, 64:128] = (Wr * sc[None, :])[:, perm]
    Wt[:, 128:192] = ((Wd - Wr) * sc[None, :])[:, perm]
    brow = np.zeros((TW,), np.float64)
    brow[0:64] = (bl * sc)[perm]
    brow[64:128] = (br * sc)[perm]
    brow[128:192] = ((b + bd - br) * sc)[perm]
    inva = 1.0 / scp
    return dict(Wt=Wt.astype(np.float32),
                brow=brow.astype(np.float32)[None, :],
                inva=inva, perm=perm, p_cnt=p_cnt)


def dummy_rows_bf16(p_cnt):
    """Table rows for dummy-zero (all 0) and dummy-neg nodes."""
    import ml_dtypes
    d = np.zeros((2, H), dtype=np.float32)
    if p_cnt > 0:
        d[1, 0] = -NEGBIG       # pos col: +|v+r| + 1.5v => -0.5*NEGBIG
    else:
        d[1, 0] = NEGBIG        # neg col: -|v+r| - 1.5v => -2.5*NEGBIG
    return d.astype(ml_dtypes.bfloat16)


# ------------------------------------------------------------- bass builder

def build_bass(cfg):
    import sys
    sys.path.insert(0, "/opt/trn_rl_repo")
    import concourse.bass as bass
    import concourse.bacc as bacc
    import concourse.mybir as mybir
    import concourse.tile as tile
    from concourse import library_config
    from concourse.masks import make_identity
    from contextlib import ExitStack

    f32 = mybir.dt.float32
    bf16 = mybir.dt.bfloat16
    i16 = mybir.dt.int16
    X = mybir.AxisListType.X
    ADD = mybir.AluOpType.add
    SUB = mybir.AluOpType.subtract
    MULT = mybir.AluOpType.mult
    MAX = mybir.AluOpType.max
    AF = mybir.ActivationFunctionType

    NT, NLOC, NPAD, F_IN, SD = cfg.NT, cfg.NLOC, cfg.NPAD, cfg.F_IN, cfg.SD
    d_t, offs = cfg.d_t, cfg.offs
    RT = NPAD // 4              # pack-table rows

    nc = bacc.Bacc("TRN2", target_bir_lowering=False, num_devices=NCORES,
                   num_swdge_queues=NQ)

    xT_in = nc.dram_tensor("xT_loc", [F_IN, NLOC], f32, kind="ExternalInput")
    gidx_in = nc.dram_tensor("gidx", [128, 8 * cfg.GCOLS], i16,
                             kind="ExternalInput")
    msk_in = nc.dram_tensor("msks", [3, P, cfg.GCOLS], mybir.dt.uint8,
                            kind="ExternalInput")
    wt1_in = nc.dram_tensor("wt1", [F_IN, TW], f32, kind="ExternalInput")
    brow1_in = nc.dram_tensor("brow1", [1, TW], f32, kind="ExternalInput")
    wt2_in = nc.dram_tensor("wt2", [H + 1, TW], f32, kind="ExternalInput")
    dummy1_in = nc.dram_tensor("dummy1", [2, H], bf16, kind="ExternalInput")
    dummy2_in = nc.dram_tensor("dummy2", [2, H], bf16, kind="ExternalInput")
    w3_in = nc.dram_tensor("w3rep", [P, H], f32, kind="ExternalInput")
    out3 = nc.dram_tensor("out3", [P, NT], f32, kind="ExternalOutput")

    rg = [list(range(NCORES))]

    with ExitStack() as ctx:
        tc = ctx.enter_context(tile.TileContext(nc))
        nc.gpsimd.load_library(library_config.mlp)
        dram = ctx.enter_context(tc.tile_pool(name="dram", bufs=1,
                                              space="DRAM"))
        xlb_loc = [dram.tile([NLOC, H], bf16, name=f"xlb{l}_loc")
                   for l in (1, 2)]
        xlb_full = [dram.tile([NPAD, H], bf16, name=f"xlb{l}_full",
                              addr_space="Shared") for l in (1, 2)]
        xrd_loc = [dram.tile([P, NT, 128], f32, name=f"xrd{l}_loc")
                   for l in (1, 2)]

        const = ctx.enter_context(tc.tile_pool(name="const", bufs=1))
        ident = const.tile([P, P], f32)
        make_identity(nc, ident[:, :])
        ones1 = const.tile([1, P], f32)
        nc.vector.memset(ones1[:, :], 1.0)
        wt1_s = const.tile([F_IN, TW], f32)
        nc.sync.dma_start(wt1_s[:, :], wt1_in[:, :])
        brow1_s = const.tile([1, TW], f32)
        nc.sync.dma_start(brow1_s[:, :], brow1_in[:, :])
        wt2_s = const.tile([H + 1, TW], f32)
        nc.sync.dma_start(wt2_s[:, :], wt2_in[:, :])
        dum_s = []
        for l, dum_i in enumerate([dummy1_in, dummy2_in]):
            dm = const.tile([2, H], bf16, name=f"dum{l}_s")
            nc.sync.dma_start(dm[:, :], dum_i[:, :])
            dum_s.append(dm)
        w3_s = const.tile([P, H], f32)
        nc.sync.dma_start(w3_s[:, :], w3_in[:, :])
        out3_s = const.tile([P, NT], f32)
        gidx_s = const.tile([128, 8 * cfg.GCOLS], i16)
        nc.sync.dma_start(gidx_s[:, :], gidx_in[:, :])
        msk_s = const.tile([P, 3, cfg.GCOLS], mybir.dt.uint8)
        nc.sync.dma_start(msk_s[:, :, :],
                          msk_in[:, :, :].rearrange("m p c -> p m c"))

        psT = ctx.enter_context(tc.tile_pool(name="psT", bufs=2, space="PSUM"))
        psM = ctx.enter_context(tc.tile_pool(name="psM", bufs=3, space="PSUM"))
        tp = ctx.enter_context(tc.tile_pool(name="tp", bufs=3))
        cp = ctx.enter_context(tc.tile_pool(name="cp", bufs=2))
        cw = ctx.enter_context(tc.tile_pool(name="cw", bufs=2))
        gp = ctx.enter_context(tc.tile_pool(name="gp", bufs=cfg.gt_bufs))
        hp = ctx.enter_context(tc.tile_pool(name="hp", bufs=2))
        wp = ctx.enter_context(tc.tile_pool(name="wp", bufs=2))

        qctr = [0]

        def transform_chunk_tail(ci, ot_chunk, layer):
            """ot_chunk [P, T_c, TW] f32 -> xlb/xrd writes for chunk ci."""
            t0, t1 = cfg.chunks[ci]
            T_c = t1 - t0
            xlb_c = cp.tile([P, T_c, H], bf16, tag="xlb_c")
            nc.scalar.copy(xlb_c[:, :, :], ot_chunk[:, :, 0:64])
            nc.sync.dma_start(
                xlb_loc[layer][t0 * P:t1 * P, :].rearrange(
                    "(t p) c -> p t c", p=P),
                xlb_c[:, :, :])
            nc.sync.dma_start(xrd_loc[layer][:, t0:t1, :],
                              ot_chunk[:, :, 64:TW])

        # ---- phase T1: layer-1 transforms from xT (pre-transposed on host)
        for ci, (c0, c1) in enumerate(cfg.chunks):
            T_c = c1 - c0
            ot_chunk = cp.tile([P, T_c, TW], f32, tag="ot")
            t = c0
            while t < c1:
                q = min(4, c1 - t)
                xst = tp.tile([F_IN, 4 * P], f32, tag="lhsT")
                nc.sync.dma_start(xst[:, 0:q * P],
                                  xT_in[:, t * P:(t + q) * P])
                for i in range(q):
                    pm = psM.tile([P, TW], f32, tag="pm")
                    nc.tensor.matmul(pm[:, :], lhsT=xst[:, i * P:(i + 1) * P],
                                     rhs=wt1_s[:, :], start=True, stop=False)
                    nc.tensor.matmul(pm[:, :], lhsT=ones1[:, :],
                                     rhs=brow1_s[:, :], start=False, stop=True)
                    nc.scalar.copy(ot_chunk[:, t + i - c0, :], pm[:, :])
                t += q
            transform_chunk_tail(ci, ot_chunk, 0)

        def finish_layer_tables(layer):
            r0 = cfg.DZ_LP
            nc.sync.dma_start(xlb_loc[layer][r0:r0 + 2, :], dum_s[layer][:, :])
            nc.gpsimd.collective_compute(
                "AllGather", mybir.AluOpType.bypass, replica_groups=rg,
                ins=[xlb_loc[layer][:, :].opt()],
                outs=[xlb_full[layer][:, :].opt()])

        finish_layer_tables(0)

        # ---- edge phases
        def edge_phase(layer, p_cnt):
            tblv = xlb_full[layer][:, :].rearrange("(r f) c -> r (f c)", f=4)
            xrd_tiles = {}

            def load_xrd(cj):
                if cj < len(cfg.chunks) and cj not in xrd_tiles:
                    b0, b1 = cfg.chunks[cj]
                    xt = cp.tile([P, b1 - b0, 128], f32, tag="xrd")
                    nc.sync.dma_start(xt[:, :, :],
                                      xrd_loc[layer][:, b0:b1, :])
                    xrd_tiles[cj] = xt

            for ci, (c0, c1) in enumerate(cfg.chunks):
                T_c = c1 - c0
                stl = cfg.plan[ci]
                S = stl[0]["S"]
                U = stl[0]["U"]
                load_xrd(ci)
                load_xrd(ci + 1)
                xrd = xrd_tiles.pop(ci)
                if S == 1:
                    xrd_sv = [xrd[:, :, :]]
                else:
                    xrd_sv = [xrd[:, :, :].rearrange(
                        "p (u s) c -> p s u c", s=S)[:, si, :, :]
                        for si in range(S)]

                den = cw.tile([P, T_c], f32, tag="den")
                emax = cw.tile([P, T_c], f32, tag="emax")
                numer = cw.tile([P, T_c, H], f32, tag="numer")

                def dview(full, si, u0, u1):
                    if S == 1:
                        return full[:, u0:u1]
                    return full[:, :].rearrange(
                        "p (u s) -> p s u", s=S)[:, si, u0:u1]

                def nview(si, u0, u1):
                    if S == 1:
                        return numer[:, u0:u1, :]
                    return numer[:, :, :].rearrange(
                        "p (u s) h -> p s u h", s=S)[:, si, u0:u1, :]

                # gather per segment (uniform [P, GSEG, 4, H] ring tiles)
                gts = {}
                for si, st in enumerate(stl):
                    for (a, b, goff) in st["segs"]:
                        ni = 128 * (b - a)
                        gt = gp.tile([P, GSEG, 4, H], bf16, tag="gt")
                        nc.gpsimd.dma_gather(
                            out_ap=gt[:, 0:b - a, :, :].rearrange(
                                "p s f c -> p s (f c)"),
                            in_ap=tblv,
                            idxs_ap=gidx_s[:, 8 * goff:8 * (goff + b - a)],
                            num_idxs=ni, num_idxs_reg=ni,
                            elem_size=4 * H,
                            queue_num=qctr[0] % NQ, single_packet=False)
                        qctr[0] += 1
                        gts[(si, a)] = gt

                for si, st in enumerate(stl):
                    SDs = st["SDs"]
                    w = wp.tile([P, SDs, WPAD], bf16, tag=f"w{si}",
                                name=f"w{si}")
                    for (a, b, goff) in st["segs"]:
                        gt = gts.pop((si, a))
                        nc.scalar.copy(w[:, a:b, 0:H], gt[:, 0:b - a, 0, :])
                        for k in (1, 2, 3):
                            mv = msk_s[:, k - 1, goff:goff + (b - a)]
                            nc.vector.copy_predicated(
                                w[:, a:b, 0:H],
                                mv.unsqueeze(2).to_broadcast([P, b - a, H]),
                                gt[:, 0:b - a, k, :])
                    e = cw.tile([P, SDs], f32, tag=f"e{si}", name=f"e{si}")
                    en = cw.tile([P, SDs], f32, tag=f"en{si}", name=f"en{si}")
                    ex = cw.tile([P, SDs], bf16, tag=f"ex{si}", name=f"ex{si}")
                    for (u0, u1) in st["runs"]:
                        d = st["ds"][u0]
                        R = u1 - u0
                        a0, a1 = st["soffs"][u0], st["soffs"][u1]
                        wr = w[:, a0:a1, 0:H].rearrange(
                            "p (r d) c -> p r d c", d=d)
                        er = e[:, a0:a1].rearrange("p (r d) -> p r d", d=d)
                        enr = en[:, a0:a1].rearrange("p (r d) -> p r d", d=d)
                        exr = ex[:, a0:a1].rearrange("p (r d) -> p r d", d=d)
                        emaxr = dview(emax, si, u0, u1)
                        denr = dview(den, si, u0, u1)
                        # w += xr (broadcast over d)
                        nc.vector.tensor_tensor(
                            wr, wr,
                            xrd_sv[si][:, u0:u1, 0:H].unsqueeze(2)
                            .to_broadcast([P, R, d, H]), ADD)
                        # e = sum_pos|w| - sum_neg|w| + 1.5(sum_pos w - sum_neg w)
                        if p_cnt == 0:
                            nc.vector.tensor_reduce(
                                er, wr[:, :, :, 0:H], X, ADD,
                                apply_absolute_value=True, negate=True)
                            nc.vector.tensor_reduce(
                                enr, wr[:, :, :, 0:H], X, ADD)
                            nc.vector.scalar_tensor_tensor(
                                er, enr, -1.5, er, MULT, ADD)
                        else:
                            nc.vector.tensor_reduce(
                                er, wr[:, :, :, 0:p_cnt], X, ADD,
                                apply_absolute_value=True)
                            nc.vector.tensor_reduce(
                                enr, wr[:, :, :, 0:p_cnt], X, ADD)
                            nc.vector.scalar_tensor_tensor(
                                er, enr, 1.5, er, MULT, ADD)
                            if p_cnt < H:
                                nc.vector.tensor_reduce(
                                    enr, wr[:, :, :, p_cnt:H], X, ADD,
                                    apply_absolute_value=True)
                                nc.vector.tensor_tensor(er, er, enr, SUB)
                                nc.vector.tensor_reduce(
                                    enr, wr[:, :, :, p_cnt:H], X, ADD)
                                nc.vector.scalar_tensor_tensor(
                                    er, enr, -1.5, er, MULT, ADD)
                        # softmax over d (negate=True gives -max directly)
                        nc.vector.tensor_reduce(
                            emaxr, er, X, MAX, negate=True)
                        nc.vector.tensor_tensor(
                            er, er,
                            emaxr.unsqueeze(2).to_broadcast([P, R, d]), ADD)
                        nc.scalar.activation(exr, er, AF.Exp)
                        nc.vector.tensor_reduce(denr, exr, X, ADD)
                        # weighted sum: w *= ex ; reduce over d
                        nc.vector.tensor_tensor(
                            wr, wr,
                            exr.unsqueeze(3).to_broadcast([P, R, d, H]),
                            MULT)
                        nc.vector.tensor_reduce(
                            nview(si, u0, u1),
                            wr.transpose([0, 1, 3, 2]), X, ADD)
                # h = relu(numer / den + xdm)   [P, T_c, 64]
                rden = cw.tile([P, T_c], f32, tag="rden")
                nc.vector.reciprocal(rden[:, :], den[:, :])
                nc.vector.tensor_tensor(
                    numer[:, :, :], numer[:, :, :],
                    rden[:, :].unsqueeze(2).to_broadcast([P, T_c, H]), MULT)
                nc.vector.tensor_tensor(
                    numer[:, :, :], numer[:, :, :], xrd[:, :, 64:128], ADD)
                h = hp.tile([P, T_c, H], f32, tag="h")
                nc.scalar.activation(h[:, :, :], numer[:, :, :], AF.Relu)

                if layer == 0:
                    # layer-2 transform for this chunk's tiles
                    ot_chunk = cp.tile([P, T_c, TW], f32, tag="ot2")
                    for t in range(c0, c1):
                        pt = psT.tile([H, P], f32, tag="pt")
                        nc.tensor.transpose(pt[:, :], h[:, t - c0, :],
                                            ident[:, :])
                        hT = tp.tile([H + 1, P], f32, tag="hT")
                        nc.scalar.copy(hT[0:H, :], pt[:, :])
                        nc.vector.memset(hT[H:H + 1, :], 1.0)
                        pm = psM.tile([P, TW], f32, tag="pm2")
                        nc.tensor.matmul(pm[:, :], lhsT=hT[:, :],
                                         rhs=wt2_s[:, :], start=True,
                                         stop=True)
                        nc.scalar.copy(ot_chunk[:, t - c0, :], pm[:, :])
                    transform_chunk_tail(ci, ot_chunk, 1)
                else:
                    # out3 column = sum_h h * w3 (product reuses numer)
                    nc.vector.tensor_tensor(
                        numer[:, :, :], h[:, :, :],
                        w3_s[:, :].unsqueeze(1).to_broadcast([P, T_c, H]),
                        MULT)
                    nc.vector.tensor_reduce(out3_s[:, c0:c1], numer[:, :, :],
                                            X, ADD)

            if layer == 0:
                finish_layer_tables(1)

        edge_phase(0, cfg.p1)
        edge_phase(1, cfg.p2)

        nc.vector.tensor_scalar(out3_s[:, :], out3_s[:, :], float(cfg.b3val),
                                None, ADD)
        nc.sync.dma_start(out3[:, :], out3_s[:, :])

    nc.finalize()
    return nc


# ------------------------------------------------------------------ kernel

def make_inputs_and_cfg(inputs, N, F_IN, NLOC, **cfg_kw):
    g = prep_graph(inputs["edge_index"], N, NLOC)
    w1 = prep_layer_weights(inputs["Wl1"], inputs["bl1"], inputs["Wr1"],
                            inputs["br1"], inputs["att1"], inputs["b1"],
                            inputs["Wd1"], inputs["bd1"])
    w2 = prep_layer_weights(inputs["Wl2"], inputs["bl2"], inputs["Wr2"],
                            inputs["br2"], inputs["att2"], inputs["b2"],
                            inputs["Wd2"], inputs["bd2"],
                            in_perm=w1["perm"], in_scale=w1["inva"])
    x = np.ascontiguousarray(np.asarray(inputs["x"], np.float32))
    W3p = np.asarray(inputs["W3"], np.float64)[w2["perm"]] * \
        w2["inva"][:, None]                                     # [H, 1]
    b3val = float(np.asarray(inputs["b3"], np.float32)[0])
    cfg = Cfg(N, F_IN, NLOC, g["d_t"], g["offs"], w1["p_cnt"], w2["p_cnt"],
              b3val, **cfg_kw)

    w3rep = np.broadcast_to(W3p[:, 0][None, :].astype(np.float32),
                            (P, H)).copy()
    d1 = dummy_rows_bf16(w1["p_cnt"])
    d2 = dummy_rows_bf16(w2["p_cnt"])
    # wt2 with bias folded as row H (lhsT row of ones)
    wt2f = np.concatenate([w2["Wt"], w2["brow"]], axis=0)

    in_maps = []
    order = g["order"]
    for c in range(NCORES):
        gidx, msks = build_gidx_masks(cfg, g["esrc"][c])
        ii = np.arange(c, N, NCORES)
        lp = ii // NCORES
        x_loc = np.zeros((NLOC, F_IN), dtype=np.float32)
        x_loc[lp] = x[order[ii]]
        in_maps.append({
            "xT_loc": np.ascontiguousarray(x_loc.T),
            "gidx": gidx, "msks": msks,
            "wt1": w1["Wt"], "brow1": w1["brow"],
            "wt2": wt2f,
            "dummy1": d1, "dummy2": d2,
            "w3rep": w3rep,
        })
    return cfg, in_maps, g


def unshard_output(results, g, N, NLOC):
    out = np.zeros((N, 1), dtype=np.float32)
    order = g["order"]
    for c in range(NCORES):
        o = np.asarray(results[c]["out3"])          # [128, NT]
        ii = np.arange(c, N, NCORES)
        lp = ii // NCORES
        out[order[ii], 0] = o[lp % P, lp // P]
    return out


def kernel(**inputs):
    import sys
    sys.path.insert(0, "/opt/trn_rl_repo")
    from concourse.bass_utils import run_bass_kernel_spmd
    N, F_IN, NLOC = 100000, 128, 12544
    cfg, in_maps, g = make_inputs_and_cfg(inputs, N, F_IN, NLOC)
    nc = build_bass(cfg)
    res = run_bass_kernel_spmd(nc, in_maps, core_ids=list(range(NCORES)))
    return unshard_output(res.results, g, N, NLOC)
